# revision 6
# baseline (speedup 1.0000x reference)
"""AttnBlock6 Trainium2 kernel (Bass/Tile, 8 NeuronCores).

Math (per sample):
  xn = GroupNorm1(x);  q,k,v = 1x1conv(xn);  wm = softmax(qf^T kf / 256)
  hp = vf @ wm^T   (patch attention over 196-wide chunks, contraction 65536)
  pooled 8x8 -> qg,kg,vg [64,3136]; wg = softmax(qg^T kg / 8); hg = vg @ wg^T
  out = x + proj( 0.75*hp + 0.25*up8(hg) )

Distribution: cores 0-3 -> sample 0, cores 4-7 -> sample 1; each core owns 112
image rows (= 256 patch chunks, 784 pooled positions). Two tiny collectives
per 4-core group: AllReduce of the 196x196 gram matrix, AllGather of pooled x.

Host-side folding: per-sample GroupNorm mean/inv-std are computed on the host
(O(N) numpy) and folded into per-sample q/k/v weight matrices, so the device
never materializes xn. Softmax scales (1/256, 1/8) and the 0.75/0.25 combine
weights are folded into weights/normalizers. v-bias contributes exactly c_v to
the combined h (softmax rows sum to 1), folded in once via the hg term.

Device layout: partition p = 64*half + channel; free = within-half position
(row-major over the core's 56-row half). Block-diagonal duplicated weights
give full K=128 matmul contractions.
"""

import numpy as np

import concourse.bacc as bacc
import concourse.bass as bass
import concourse.tile as tile
from concourse import mybir
from concourse.bass_utils import run_bass_kernel_spmd
from concourse.masks import make_identity

C = 64
SIZE = 448
P2 = 196
TG = 56
POOL = 8
EPS = 1e-5
B = 2

NCORES = 8
GROUP = 4                  # cores per sample
ROWS = SIZE // GROUP       # 112 image rows per core
F = ROWS * SIZE // 2       # 25088 free elems per half
SB = 3136                  # super-block: 7 rows = 16 chunks per half
NSB = F // SB              # 8
PPL = 784                  # pooled positions per core (2 * 7 * 56)
PG = 3136                  # pooled positions per sample (56*56)
KT = 112                   # pooled k-tile (fits rank-contiguous AGed layout)
NKT = PG // KT             # 28

FP32 = mybir.dt.float32
BF16 = mybir.dt.bfloat16
AF = mybir.ActivationFunctionType
ALU = mybir.AluOpType

_CACHE = {}


def _build_bass():
    nc = bacc.Bacc("TRN2", target_bir_lowering=False, debug=False,
                   num_devices=NCORES)

    x2 = nc.dram_tensor("x2", [128, F], FP32, kind="ExternalInput")
    w2q = nc.dram_tensor("w2q", [128, 128], BF16, kind="ExternalInput")
    w2k = nc.dram_tensor("w2k", [128, 128], BF16, kind="ExternalInput")
    w2v = nc.dram_tensor("w2v", [128, 128], BF16, kind="ExternalInput")
    w2p = nc.dram_tensor("w2p", [128, 128], BF16, kind="ExternalInput")
    cq2 = nc.dram_tensor("cq2", [128, 1], FP32, kind="ExternalInput")
    ck2 = nc.dram_tensor("ck2", [128, 1], FP32, kind="ExternalInput")
    cv2 = nc.dram_tensor("cv2", [128, 1], FP32, kind="ExternalInput")
    agq = nc.dram_tensor("agq", [64, 64], BF16, kind="ExternalInput")
    agk = nc.dram_tensor("agk", [64, 64], BF16, kind="ExternalInput")
    agv = nc.dram_tensor("agv", [64, 64], BF16, kind="ExternalInput")
    cgq = nc.dram_tensor("cgq", [64, 1], FP32, kind="ExternalInput")
    cgk = nc.dram_tensor("cgk", [64, 1], FP32, kind="ExternalInput")
    y2 = nc.dram_tensor("y2", [128, F], FP32, kind="ExternalOutput")

    with tile.TileContext(nc) as tc:
        _emit(nc, tc, x2, w2q, w2k, w2v, w2p, cq2, ck2, cv2,
              agq, agk, agv, cgq, cgk, y2)
    nc.finalize()
    return nc


def _ap(t, dims, offset=0):
    """Manual strided view of a DRAM tile: dims = [[step, count], ...]."""
    return bass.AP(tensor=t.tensor, offset=t.offset + offset, ap=list(dims))


def _emit(nc, tc, x2, w2q, w2k, w2v, w2p, cq2, ck2, cv2,
          agq, agk, agv, cgq, cgk, y2):
    import contextlib
    ctx = contextlib.ExitStack()
    with ctx:
        singles = ctx.enter_context(tc.tile_pool(name="singles", bufs=1))
        dram = ctx.enter_context(tc.tile_pool(name="dram", bufs=1, space="DRAM"))

        # ---- load constants ----
        W2q = singles.tile([128, 128], BF16)
        W2k = singles.tile([128, 128], BF16)
        W2v = singles.tile([128, 128], BF16)
        W2p = singles.tile([128, 128], BF16)
        for t, src in ((W2q, w2q), (W2k, w2k), (W2v, w2v), (W2p, w2p)):
            nc.sync.dma_start(out=t[:], in_=src[:, :])
        Cq = singles.tile([128, 1], FP32)
        Ck = singles.tile([128, 1], FP32)
        Cv = singles.tile([128, 1], FP32)
        for t, src in ((Cq, cq2), (Ck, ck2), (Cv, cv2)):
            nc.sync.dma_start(out=t[:], in_=src[:, :])
        Agq = singles.tile([64, 64], BF16)
        Agk = singles.tile([64, 64], BF16)
        Agv = singles.tile([64, 64], BF16)
        for t, src in ((Agq, agq), (Agk, agk), (Agv, agv)):
            nc.sync.dma_start(out=t[:], in_=src[:, :])
        Cgq = singles.tile([64, 1], FP32)
        Cgk = singles.tile([64, 1], FP32)
        for t, src in ((Cgq, cgq), (Cgk, cgk)):
            nc.sync.dma_start(out=t[:], in_=src[:, :])
        ident = singles.tile([98, 98], FP32)
        make_identity(nc, ident)

        # ---- persistent SBUF ----
        X = singles.tile([128, F], FP32)          # raw x, 2-half layout
        poolw = singles.tile([128, 56, 56], BF16)  # w-pooled sums
        pool2 = singles.tile([128, 392], FP32)     # 8x8-pooled sums (2-half)
        pool2b = singles.tile([128, 392], BF16)

        # DRAM scratch / collective buffers
        cc_m_in = dram.tile([P2, P2], FP32)
        cc_m_out = dram.tile([P2, P2], FP32)
        cc_p_in = dram.tile([64, PPL], BF16)
        cc_p_out = dram.tile([GROUP * 64, PPL], BF16)
        hg_dram = dram.tile([64, PPL], FP32)
        rec_dram = dram.tile([1, PPL], FP32)

        # =========== PASS A ===========
        mps_a = None
        mps_b = None
        with (
            tc.tile_pool(name="apsum", bufs=2, space="PSUM") as apsum,
            tc.tile_pool(name="mpsum", bufs=1, space="PSUM") as mpsum,
            tc.tile_pool(name="aqk", bufs=3) as aqk,
            tc.tile_pool(name="axb", bufs=2) as axb,
        ):
            mps_a = mpsum.tile([98, P2], FP32, tag="mps_a")
            mps_b = mpsum.tile([98, P2], FP32, tag="mps_b")
            nblk = 0
            for sb in range(NSB):
                o = sb * SB
                nc.sync.dma_start(out=X[:, o:o + SB], in_=x2[:, o:o + SB])
                xb = axb.tile([128, SB], BF16, tag="xb")
                nc.vector.tensor_copy(out=xb[:], in_=X[:, o:o + SB])
                # q/k projections + gram accumulation, per 392 cols
                for j in range(SB // 392):
                    a = j * 392
                    qp = apsum.tile([128, 392], FP32, tag="qp")
                    nc.tensor.matmul(qp[:], W2q[:], xb[:, a:a + 392])
                    qs = aqk.tile([128, 392], BF16, tag="qs")
                    nc.scalar.activation(qs[:], qp[:], AF.Identity,
                                         bias=Cq[:, 0:1])
                    kp = apsum.tile([128, 392], FP32, tag="kp")
                    nc.tensor.matmul(kp[:], W2k[:], xb[:, a:a + 392])
                    ks = aqk.tile([128, 392], BF16, tag="ks")
                    nc.scalar.activation(ks[:], kp[:], AF.Identity,
                                         bias=Ck[:, 0:1])
                    for cc in range(2):
                        q1 = qs[:, cc * 196:cc * 196 + 196]
                        k1 = ks[:, cc * 196:cc * 196 + 196]
                        first = nblk == 0
                        last = nblk == 127
                        nc.tensor.matmul(mps_a[:], q1[:, 0:98], k1,
                                         start=first, stop=last)
                        nc.tensor.matmul(mps_b[:], q1[:, 98:196], k1,
                                         start=first, stop=last)
                        nblk += 1
                # pooling: w-dir (factor 8) for 7 rows at once
                xv = xb[:].rearrange("p (r c e) -> p r c e", r=7, e=8)
                pw = poolw[:, sb * 7:sb * 7 + 7, :]
                nc.vector.tensor_tensor(pw, xv[:, :, :, 0], xv[:, :, :, 1],
                                        ALU.add)
                for e in range(2, 8):
                    nc.vector.tensor_tensor(pw, pw, xv[:, :, :, e], ALU.add)
            # h-pool (factor 8 over rows)
            pv = poolw[:].rearrange("p (r e) c -> p r e c", e=8)
            p2v = pool2[:].rearrange("p (r c) -> p r c", r=7)
            nc.vector.tensor_tensor(p2v, pv[:, :, 0], pv[:, :, 1], ALU.add)
            for e in range(2, 8):
                nc.vector.tensor_tensor(p2v, p2v, pv[:, :, e], ALU.add)
            nc.vector.tensor_copy(out=pool2b[:], in_=pool2[:])

            # evict gram psum, ship collectives
            msb_a = aqk.tile([98, P2], FP32, tag="msb_a")
            msb_b = aqk.tile([98, P2], FP32, tag="msb_b")
            nc.scalar.copy(msb_a[:], mps_a[:])
            nc.scalar.copy(msb_b[:], mps_b[:])
            nc.gpsimd.dma_start(out=cc_m_in[0:98, :], in_=msb_a[:])
            nc.gpsimd.dma_start(out=cc_m_in[98:196, :], in_=msb_b[:])
            nc.gpsimd.dma_start(
                out=_ap(cc_p_in, [[392, 2], [PPL, 64], [1, 392]]),
                in_=pool2b[:])

        nc.gpsimd.collective_compute(
            "AllReduce", ALU.add,
            replica_groups=[[0, 1, 2, 3], [4, 5, 6, 7]],
            ins=[cc_m_in.opt()], outs=[cc_m_out.opt()])
        nc.gpsimd.collective_compute(
            "AllGather", ALU.bypass,
            replica_groups=[[0, 1, 2, 3], [4, 5, 6, 7]],
            ins=[cc_p_in.opt()], outs=[cc_p_out.opt()])

        # =========== local softmax of wm; build wmT (bf16, x0.75) ===========
        wmT_a = singles.tile([98, P2], BF16)   # k in [0,98)
        wmT_b = singles.tile([98, P2], BF16)   # k in [98,196)
        with (
            tc.tile_pool(name="wmp", bufs=1) as wmp,
            tc.tile_pool(name="wmps", bufs=2, space="PSUM") as wmps,
        ):
            m0 = wmp.tile([98, P2], FP32, tag="m0")
            m1 = wmp.tile([98, P2], FP32, tag="m1")
            nc.sync.dma_start(out=m0[:], in_=cc_m_out[0:98, :])
            nc.sync.dma_start(out=m1[:], in_=cc_m_out[98:196, :])
            e0 = wmp.tile([98, P2], FP32, tag="e0")
            e1 = wmp.tile([98, P2], FP32, tag="e1")
            nc.scalar.activation(e0[:], m0[:], AF.Exp)
            nc.scalar.activation(e1[:], m1[:], AF.Exp)
            s0 = wmp.tile([98, 1], FP32, tag="s0")
            s1 = wmp.tile([98, 1], FP32, tag="s1")
            nc.vector.reduce_sum(out=s0[:], in_=e0[:], axis=mybir.AxisListType.X)
            nc.vector.reduce_sum(out=s1[:], in_=e1[:], axis=mybir.AxisListType.X)
            nc.vector.reciprocal(out=s0[:], in_=s0[:])
            nc.vector.reciprocal(out=s1[:], in_=s1[:])
            nc.vector.tensor_scalar_mul(s0[:], s0[:], 0.75)
            nc.vector.tensor_scalar_mul(s1[:], s1[:], 0.75)
            wn0 = wmp.tile([98, P2], FP32, tag="wn0")
            wn1 = wmp.tile([98, P2], FP32, tag="wn1")
            nc.vector.tensor_scalar(wn0[:], e0[:], s0[:, 0:1], None, op0=ALU.mult)
            nc.vector.tensor_scalar(wn1[:], e1[:], s1[:, 0:1], None, op0=ALU.mult)
            for (dst, srcs) in ((wmT_a, (wn0[:, 0:98], wn1[:, 0:98])),
                                (wmT_b, (wn0[:, 98:196], wn1[:, 98:196]))):
                for half, src in enumerate(srcs):
                    tp = wmps.tile([98, 98], FP32, tag="tp")
                    nc.tensor.transpose(tp[:], src, ident[:])
                    nc.scalar.copy(dst[:, half * 98:half * 98 + 98], tp[:])

        # =========== global pooled attention ===========
        hgc2 = singles.tile([128, 392], FP32)
        with (
            tc.tile_pool(name="gp", bufs=1) as gp,
            tc.tile_pool(name="gwge", bufs=3) as gwge,
            tc.tile_pool(name="gps", bufs=2, space="PSUM") as gps,
            tc.tile_pool(name="ghps", bufs=1, space="PSUM") as ghps,
        ):
            kgsrc = gp.tile([64, PG], BF16, tag="kgsrc")
            nc.sync.dma_start(
                out=kgsrc[:].rearrange("c (r i) -> c r i", r=GROUP),
                in_=_ap(cc_p_out, [[PPL, 64], [64 * PPL, GROUP], [1, PPL]]))
            # qg for my local positions (pooled x back in [64, 784] layout)
            qsrc = gp.tile([64, PPL], BF16, tag="qsrc")
            nc.sync.dma_start(out=qsrc[:], in_=cc_p_in[:, :])
            qgb = gp.tile([64, PPL], BF16, tag="qgb")
            for h in range(2):
                qp = gps.tile([64, 392], FP32, tag="gsm")
                nc.tensor.matmul(qp[:], Agq[:],
                                 qsrc[:, h * 392:h * 392 + 392])
                nc.scalar.activation(qgb[:, h * 392:h * 392 + 392], qp[:],
                                     AF.Identity, bias=Cgq[:, 0:1])
            # kg for all positions of the sample
            kgb = gp.tile([64, PG], BF16, tag="kgb")
            for j in range(2 * GROUP):
                kp = gps.tile([64, 392], FP32, tag="gsm")
                nc.tensor.matmul(kp[:], Agk[:], kgsrc[:, j * 392:(j + 1) * 392])
                nc.scalar.activation(kgb[:, j * 392:(j + 1) * 392], kp[:],
                                     AF.Identity, bias=Cgk[:, 0:1])
            # vgT tiles [112, 65] (last col = ones for the denominator row)
            vgT = gp.tile([112, NKT, 65], BF16, tag="vgT")
            for kt in range(NKT):
                vp = gps.tile([112, 64], FP32, tag="gsm")
                nc.tensor.matmul(vp[:], kgsrc[:, kt * KT:(kt + 1) * KT], Agv[:])
                nc.scalar.copy(vgT[:, kt, 0:64], vp[:])
                nc.vector.memset(vgT[:, kt, 64:65], 1.0)
            # stream k-tiles: wgT logits -> exp -> accumulate hg + denom
            hg0 = ghps.tile([65, 392], FP32, tag="hg0")
            hg1 = ghps.tile([65, 392], FP32, tag="hg1")
            for kt in range(NKT):
                wgp = gps.tile([112, PPL], FP32, tag="wgp")
                for n in range(2):
                    nc.tensor.matmul(wgp[:, n * 392:n * 392 + 392],
                                     kgb[:, kt * KT:(kt + 1) * KT],
                                     qgb[:, n * 392:n * 392 + 392])
                wge = gwge.tile([112, PPL], BF16, tag="wge")
                nc.scalar.activation(wge[:], wgp[:], AF.Exp)
                for n, hgp in ((0, hg0), (1, hg1)):
                    nc.tensor.matmul(hgp[:], vgT[:, kt, :],
                                     wge[:, n * 392:n * 392 + 392],
                                     start=(kt == 0), stop=(kt == NKT - 1))
            # normalize + rearrange to (half,c) layout via DRAM bounce
            hgA = gp.tile([64, PPL], FP32, tag="hgA")
            rec = gp.tile([1, PPL], FP32, tag="rec")
            for n, hgp in ((0, hg0), (1, hg1)):
                nc.scalar.copy(hgA[:, n * 392:n * 392 + 392], hgp[0:64, :])
                nc.scalar.copy(rec[:, n * 392:n * 392 + 392], hgp[64:65, :])
            nc.vector.reciprocal(out=rec[:], in_=rec[:])
            nc.vector.tensor_scalar_mul(rec[:], rec[:], 0.25)
            nc.sync.dma_start(out=hg_dram[:], in_=hgA[:])
            nc.sync.dma_start(out=rec_dram[:], in_=rec[:])
            hgB = gp.tile([128, 392], FP32, tag="hgB")
            nc.sync.dma_start(out=hgB[:],
                              in_=_ap(hg_dram, [[392, 2], [PPL, 64], [1, 392]]))
            rec2 = gp.tile([128, 392], FP32, tag="rec2")
            nc.sync.dma_start(
                out=rec2[:],
                in_=_ap(rec_dram, [[392, 2], [0, 64], [1, 392]]))
            nc.vector.tensor_tensor(hgc2[:], hgB[:], rec2[:], ALU.mult)
            nc.vector.tensor_scalar(hgc2[:], hgc2[:], Cv[:, 0:1], None,
                                    op0=ALU.add)

        # =========== PASS B ===========
        with (
            tc.tile_pool(name="bxb", bufs=2) as bxb,
            tc.tile_pool(name="bvt", bufs=4) as bvt,
            tc.tile_pool(name="bh", bufs=1) as bh,
            tc.tile_pool(name="bout", bufs=4) as bout,
            tc.tile_pool(name="bvps", bufs=2, space="PSUM") as bvps,
            tc.tile_pool(name="bhps", bufs=3, space="PSUM") as bhps,
            tc.tile_pool(name="bpps", bufs=2, space="PSUM") as bpps,
        ):
            for sb in range(NSB):
                o = sb * SB
                xb = bxb.tile([128, SB], BF16, tag="xb")
                nc.vector.tensor_copy(out=xb[:], in_=X[:, o:o + SB])
                h_sb = bh.tile([128, SB], BF16, tag="h")
                hps = []
                for t in range(SB // 392):
                    a = t * 392
                    vts = []
                    for u in range(4):
                        vp = bvps.tile([98, 128], FP32, tag="vp")
                        nc.tensor.matmul(vp[:], xb[:, a + u * 98:a + u * 98 + 98],
                                         W2v[:])
                        vt = bvt.tile([98, 128], BF16, tag="vt")
                        nc.scalar.copy(vt[:], vp[:])
                        vts.append(vt)
                    hp = bhps.tile([128, 392], FP32, tag="hp")
                    for cc in range(2):
                        nc.tensor.matmul(hp[:, cc * 196:cc * 196 + 196],
                                         vts[2 * cc][:], wmT_a[:],
                                         start=True, stop=False)
                        nc.tensor.matmul(hp[:, cc * 196:cc * 196 + 196],
                                         vts[2 * cc + 1][:], wmT_b[:],
                                         start=False, stop=True)
                    hps.append(hp)
                # combine h = hp + hgc2_upsampled, per (row x psum-tile) frag
                for r in range(7):
                    pr = (7 * sb + r) // 8
                    a = r * 448
                    while a < (r + 1) * 448:
                        t = a // 392
                        b = min((r + 1) * 448, (t + 1) * 392)
                        pc0 = (a - r * 448) // 8
                        w = (b - a) // 8
                        hgv = hgc2[:, pr * 56 + pc0:pr * 56 + pc0 + w, None] \
                            .to_broadcast([128, w, 8])
                        nc.vector.tensor_tensor(
                            h_sb[:, a:b].rearrange("p (c e) -> p c e", e=8),
                            hps[t][:, a - t * 392:b - t * 392]
                            .rearrange("p (c e) -> p c e", e=8),
                            hgv, ALU.add)
                        a = b
                # proj + residual + store, per image row
                for r in range(7):
                    a = r * 448
                    pp = bpps.tile([128, 448], FP32, tag="pp")
                    nc.tensor.matmul(pp[:], W2p[:], h_sb[:, a:a + 448])
                    ot = bout.tile([128, 448], FP32, tag="ot")
                    nc.vector.tensor_tensor(ot[:], pp[:], X[:, o + a:o + a + 448],
                                            ALU.add)
                    nc.sync.dma_start(out=y2[:, o + a:o + a + 448], in_=ot[:])


def _fold_weights(x, gn_w, gn_b, q_w, q_b, k_w, k_b, v_w, v_b, proj_w):
    """Per-sample folded weight sets for the device program."""
    bf16 = mybir.dt.np(BF16)
    outs = []
    for s in range(B):
        xs = x[s].astype(np.float64)
        m = xs.mean()
        r = 1.0 / np.sqrt(xs.var() + EPS)
        scale = (r * gn_w).astype(np.float64)           # [C]
        shift = (gn_b - m * r * gn_w).astype(np.float64)  # [C]

        def fold(w, bias):
            A = (w.astype(np.float64) * scale[None, :])
            c = w.astype(np.float64) @ shift + bias.astype(np.float64)
            return A, c

        Aq, cq = fold(q_w, q_b)
        Ak, ck = fold(k_w, k_b)
        Av, cv = fold(v_w, v_b)

        def bd(a):  # block-diag duplicated transpose [128,128]
            z = np.zeros((128, 128), np.float64)
            z[0:64, 0:64] = a.T
            z[64:128, 64:128] = a.T
            return z

        d = {
            "w2q": (bd(Aq) / 256.0).astype(bf16),
            "w2k": bd(Ak).astype(bf16),
            "w2v": bd(Av).astype(bf16),
            "w2p": bd(proj_w.astype(np.float64)).astype(bf16),
            "cq2": np.tile(cq / 256.0, 2)[:, None].astype(np.float32),
            "ck2": np.tile(ck, 2)[:, None].astype(np.float32),
            "cv2": np.tile(cv, 2)[:, None].astype(np.float32),
            "agq": (Aq.T / (64.0 * 8.0)).astype(bf16),
            "agk": (Ak.T / 64.0).astype(bf16),
            "agv": (Av.T / 64.0).astype(bf16),
            "cgq": (cq[:, None] / 8.0).astype(np.float32),
            "cgk": ck[:, None].astype(np.float32),
        }
        outs.append(d)
    return outs


def kernel(x, gn_w, gn_b, q_w, q_b, k_w, k_b, v_w, v_b, proj_w):
    x = np.asarray(x, np.float32)
    args = [np.asarray(a, np.float32) for a in
            (gn_w, gn_b, q_w, q_b, k_w, k_b, v_w, v_b, proj_w)]

    if "nc" not in _CACHE:
        _CACHE["nc"] = _build_bass()
    nc = _CACHE["nc"]

    folded = _fold_weights(x, *args)
    in_maps = []
    for core in range(NCORES):
        s, rr = divmod(core, GROUP)
        r0 = rr * ROWS
        h0 = x[s][:, r0:r0 + 56, :].reshape(64, F)
        h1 = x[s][:, r0 + 56:r0 + ROWS, :].reshape(64, F)
        m = dict(folded[s])
        m["x2"] = np.ascontiguousarray(np.concatenate([h0, h1], axis=0))
        in_maps.append(m)

    res = run_bass_kernel_spmd(nc, in_maps, core_ids=list(range(NCORES)))

    out = np.empty((B, C, SIZE, SIZE), np.float32)
    for core in range(NCORES):
        s, rr = divmod(core, GROUP)
        r0 = rr * ROWS
        yv = res.results[core]["y2"]
        out[s][:, r0:r0 + 56, :] = yv[0:64].reshape(64, 56, SIZE)
        out[s][:, r0 + 56:r0 + ROWS, :] = yv[64:128].reshape(64, 56, SIZE)
    return out
